# revision 2
# baseline (speedup 1.0000x reference)
"""Trainium2 Bass kernel v6: full softmax attention, 3-engine rebalance.

v5 -> v6 (trace-driven: ACT was 97% / DVE 108% of the 27.3us PE window,
causing ~21us of psum-recycle stalls at 269us total):
  - exp split ACT:DVE goes 24:8 -> 21:11 (DVE Schraudolph tiles)
  - reduction tree L0+L1 for chunks 0-1 moved to GPSIMD (was idle);
    chunks 2-3 stay on DVE; L2+L3 eliminated -- lr output is 4x wider
    ([128, 8*SB] per (h,sb)) and the host finishes the sum
  - warmup matmul count trimmed 14 -> 10
Per-sb budget now: ACT ~23.2us, DVE ~22.4us, GPSIMD ~24.4us (off the
PE critical path) vs PE 27.3us.
"""

import numpy as np
import ml_dtypes
from contextlib import ExitStack

import concourse.bass as bass
import concourse.bacc as bacc
import concourse.mybir as mybir
import concourse.tile as tile
from concourse.bass_utils import run_bass_kernel_spmd

B, S, H, D = 1, 4096, 16, 128
N_CORES = 8
HPC = H // N_CORES
SB = 1024
NSB = S // SB
NKT = S // 128
SCALE = float(1.0 / np.sqrt(D))
BF16 = mybir.dt.bfloat16
FP32 = mybir.dt.float32
I16 = mybir.dt.int16

SCH_SIGMA = 0.05754
SCH_A = float(SCALE * 128.0 / np.log(2.0))
SCH_B = float(128.0 * (127.0 - SCH_SIGMA))
DVE_SET = frozenset((2, 5, 8, 11, 14, 17, 20, 23, 26, 29, 31))

_CACHE = {}


def _build():
    nc = bacc.Bacc("TRN2", target_bir_lowering=False, debug=False)
    qt_d = nc.dram_tensor("qt", [HPC, 128, S], BF16, kind="ExternalInput")
    kt_d = nc.dram_tensor("kt", [HPC, 128, S], BF16, kind="ExternalInput")
    vp_d = nc.dram_tensor("vp", [HPC, 128, S], BF16, kind="ExternalInput")
    o_d = nc.dram_tensor("o", [HPC, NSB, 128, SB], FP32, kind="ExternalOutput")
    lr_d = nc.dram_tensor("lr", [HPC, NSB, 128, 8 * SB], BF16, kind="ExternalOutput")

    with ExitStack() as ctx:
        tc = ctx.enter_context(tile.TileContext(nc))
        qkv = ctx.enter_context(tc.tile_pool(name="qkv", bufs=2))
        ptp = ctx.enter_context(tc.tile_pool(name="ptp", bufs=1))
        trp = ctx.enter_context(tc.tile_pool(name="trp", bufs=1))
        drp = ctx.enter_context(tc.tile_pool(name="drp", bufs=2))

        scp = ctx.enter_context(tc.tile_pool(name="scp", bufs=3, space="PSUM"))
        otp = ctx.enter_context(tc.tile_pool(name="otp", bufs=1, space="PSUM"))

        wsrc = qkv.tile([128, 512], BF16, name="wsrc", tag="wsrc")
        nc.vector.memset(wsrc, 1.0)
        wsc = scp.tile([128, SB], FP32, name="wsc", tag="sc")
        for wi in range(10):
            nc.tensor.matmul(wsc[:, (wi % 2) * 512:(wi % 2) * 512 + 512],
                             wsrc[:, :128], wsrc, start=True, stop=True)

        deferred = []
        pvq = []
        for h in range(HPC):
            qt_s = qkv.tile([128, S], BF16, name=f"qt{h}", tag="qt")
            kt_s = qkv.tile([128, S], BF16, name=f"kt{h}", tag="kt")
            v_s = qkv.tile([128, S], BF16, name=f"v{h}", tag="v")
            if h == 0:
                # need-ordered startup: kt k-tile0 + first q half, then rest
                nc.sync.dma_start(kt_s[:, 0:128], kt_d[h][:, 0:128])
                nc.sync.dma_start(qt_s[:, 0:512], qt_d[h][:, 0:512])
                nc.sync.dma_start(kt_s[:, 128:512], kt_d[h][:, 128:512])
                nc.sync.dma_start(qt_s[:, 512:1024], qt_d[h][:, 512:1024])
                nc.sync.dma_start(v_s[:, 0:512], vp_d[h][:, 0:512])
                bounds = [512, 1024, 2048, 3072, 4096]
                for a, b in zip(bounds[:-1], bounds[1:]):
                    nc.sync.dma_start(kt_s[:, a:b], kt_d[h][:, a:b])
                    if a >= 1024:
                        nc.sync.dma_start(qt_s[:, a:b], qt_d[h][:, a:b])
                    nc.sync.dma_start(v_s[:, a:b], vp_d[h][:, a:b])
            else:
                for a, b in [(0, 1024), (1024, 2048), (2048, 3072), (3072, 4096)]:
                    nc.sync.dma_start(kt_s[:, a:b], kt_d[h][:, a:b])
                    nc.sync.dma_start(qt_s[:, a:b], qt_d[h][:, a:b])
                    nc.sync.dma_start(v_s[:, a:b], vp_d[h][:, a:b])

            for sb in range(NSB):
                q0 = sb * SB
                last = (h == HPC - 1) and (sb == NSB - 1)
                ot = otp.tile([128, SB], FP32, name=f"ot_{h}_{sb}", tag="ot")
                pt = ptp.tile([128, NKT * SB], BF16, name=f"pt_{h}_{sb}", tag="pt")
                pt_i16 = pt.bitcast(I16)
                t1 = trp.tile([128, 16 * SB], BF16, name=f"t1_{h}_{sb}", tag="t1")
                t2 = trp.tile([128, 8 * SB], BF16, name=f"t2_{h}_{sb}", tag="t2")

                def pv(j, ot=ot, pt=pt, v_s=v_s):
                    vj = v_s[:, j * 128:(j + 1) * 128]
                    pj = pt[:, j * SB:(j + 1) * SB]
                    nc.tensor.matmul(ot[:, :512], vj, pj[:, :512],
                                     start=(j == 0), stop=(j == NKT - 1))
                    nc.tensor.matmul(ot[:, 512:], vj, pj[:, 512:],
                                     start=(j == 0), stop=(j == NKT - 1))

                def l0half(c, hh2, eng, pt=pt, t1=t1):
                    # one L0 add: 4 pt tiles -> 2 t1 tiles for chunk c half hh2
                    o2 = (2 * c + hh2) * 4 * SB
                    src = pt[:, o2:o2 + 4 * SB].rearrange(
                        "p (t two q) -> p t two q", two=2, q=SB)
                    dst = t1[:, o2 // 2:o2 // 2 + 2 * SB].rearrange(
                        "p (t q) -> p t q", q=SB)
                    eng.tensor_add(dst, src[:, :, 0, :], src[:, :, 1, :])

                def l1k(k, eng, t1=t1, t2=t2):
                    # one L1 add: t1 tiles 4k..4k+3 -> t2 tiles 2k..2k+1
                    o2 = k * 4 * SB
                    src = t1[:, o2:o2 + 4 * SB].rearrange(
                        "p (t two q) -> p t two q", two=2, q=SB)
                    dst = t2[:, o2 // 2:o2 // 2 + 2 * SB].rearrange(
                        "p (t q) -> p t q", q=SB)
                    eng.tensor_add(dst, src[:, :, 0, :], src[:, :, 1, :])

                def lrq(k, h=h, sb=sb, t2=t2, last=last):
                    # DMA out lr quarter k ( = t2 tiles 2k..2k+1 )
                    cs = slice(k * 2 * SB, (k + 1) * 2 * SB)
                    if last:
                        for qq in range(2):
                            c2 = slice((2 * k + qq) * SB, (2 * k + qq + 1) * SB)
                            nc.sync.dma_start(lr_d[h, sb][:, c2], t2[:, c2])
                    else:
                        nc.sync.dma_start(lr_d[h, sb][:, cs], t2[:, cs])

                for j in range(NKT):
                    sc = scp.tile([128, SB], FP32, name=f"sc_{h}_{sb}_{j}", tag="sc")
                    kj = kt_s[:, j * 128:(j + 1) * 128]
                    nc.tensor.matmul(sc[:, :512], kj, qt_s[:, q0:q0 + 512],
                                     start=True, stop=True)
                    nc.tensor.matmul(sc[:, 512:], kj, qt_s[:, q0 + 512:q0 + SB],
                                     start=True, stop=True)
                    if j in DVE_SET:
                        nc.vector.tensor_scalar(
                            pt_i16[:, j * SB:(j + 1) * SB], sc, SCH_A, SCH_B,
                            mybir.AluOpType.mult, mybir.AluOpType.add)
                    else:
                        nc.scalar.activation(
                            pt[:, j * SB:(j + 1) * SB], sc,
                            mybir.ActivationFunctionType.Exp, scale=SCALE)

                    # tree beats (chunks 0-1 on gpsimd, 2-3 on DVE)
                    if j == 9:
                        l0half(0, 0, nc.gpsimd)
                        l0half(0, 1, nc.gpsimd)
                    elif j == 17:
                        l0half(1, 0, nc.gpsimd)
                        l0half(1, 1, nc.gpsimd)
                    elif j == 19:
                        l1k(0, nc.gpsimd)
                    elif j == 21:
                        l1k(1, nc.gpsimd)
                    elif j == 23:
                        lrq(0)
                    elif j == 25:
                        l0half(2, 0, nc.vector)
                        lrq(1)
                    elif j == 27:
                        l0half(2, 1, nc.vector)
                    elif j == 29:
                        l1k(2, nc.vector)
                    elif j == 30:
                        lrq(2)
                    if deferred and j in (1, 3, 5, 7):
                        deferred.pop(0)()

                    def pvstep(j=j, pv=pv, h=h, sb=sb, ot=ot, last=last):
                        pv(j)
                        if j == NKT - 1:
                            # sb epilogue rides with the last PV: drain ot
                            osb = drp.tile([128, SB], FP32,
                                           name=f"osb_{h}_{sb}", tag="osb")
                            for qq in range(4):
                                cs = slice(qq * SB // 4, (qq + 1) * SB // 4)
                                if last:
                                    nc.scalar.copy(osb[:, cs], ot[:, cs])
                                else:
                                    nc.vector.tensor_copy(osb[:, cs], ot[:, cs])
                                nc.sync.dma_start(o_d[h, sb][:, cs], osb[:, cs])
                    pvq.append(pvstep)
                    if len(pvq) > 3:
                        pvq.pop(0)()

                def tailc3a(l0half=l0half):
                    l0half(3, 0, nc.vector)
                def tailc3b(l0half=l0half):
                    l0half(3, 1, nc.vector)
                def tailk3(l1k=l1k):
                    l1k(3, nc.vector)
                def taillr(lrq=lrq):
                    lrq(3)
                if last:
                    while pvq:
                        pvq.pop(0)()
                    tailc3a(); tailc3b(); tailk3(); taillr()
                else:
                    deferred.extend([tailc3a, tailc3b, tailk3, taillr])
        while pvq:
            pvq.pop(0)()
        while deferred:
            deferred.pop(0)()
    nc.compile()
    return nc


def _prep_inputs(q, k, v):
    bf = ml_dtypes.bfloat16
    in_maps = []
    for c in range(N_CORES):
        hs = slice(c * HPC, (c + 1) * HPC)
        qt = np.transpose(q[:, hs, :], (1, 2, 0)).astype(bf)
        kt = np.transpose(k[:, hs, :], (1, 2, 0)).astype(bf)
        vh = np.transpose(v[:, hs, :], (1, 0, 2))
        vp = np.ascontiguousarray(
            vh.reshape(HPC, S // 128, 128, D).transpose(0, 2, 1, 3)
        ).reshape(HPC, 128, S).astype(bf)
        in_maps.append({"qt": qt, "kt": kt, "vp": vp})
    return in_maps


def kernel(q, k, v, ring_size=None, **_unused):
    q = np.asarray(q, dtype=np.float32).reshape(S, H, D)
    k = np.asarray(k, dtype=np.float32).reshape(S, H, D)
    v = np.asarray(v, dtype=np.float32).reshape(S, H, D)

    in_maps = _prep_inputs(q, k, v)
    if "nc" not in _CACHE:
        _CACHE["nc"] = _build()
    res = run_bass_kernel_spmd(_CACHE["nc"], in_maps, list(range(N_CORES))).results

    out = np.empty((B, S, H, D), np.float32)
    for c in range(N_CORES):
        o = np.asarray(res[c]["o"])
        lr = np.asarray(res[c]["lr"]).astype(np.float32)
        for hh in range(HPC):
            l = lr[hh].reshape(NSB, 128, 8, SB).sum(axis=(1, 2))
            on = o[hh] / l[:, None, :]
            out[0, :, c * HPC + hh, :] = on.transpose(0, 2, 1).reshape(S, D)
    return out


# revision 3
# speedup vs baseline: 1.2994x; 1.2994x over previous
"""Trainium2 Bass kernel v7: full softmax attention, ACT/DVE rebalance.

v6 post-mortem: GPSIMD tensor_tensor holds the SBUF port pair that DVE's
2x mode needs -- every overlapping DVE op blocked for the full gpsimd op
duration (1.2us -> 4.8us). GPSIMD offload abandoned.

v7 (vs v5 baseline, trace-driven):
  - reduction tree keeps only L0 on-chip (8 DVE adds/sb); the 16-tile
    partial sums go to HBM ([128, 16*SB] bf16 per (h,sb)) and the host
    finishes the reduction. DVE tree work drops 18.3us -> 9.8us per sb.
  - exp split ACT:DVE 24:8 -> 22:10 (ACT 26.5 -> 24.3us per sb)
  - per-sb budget: ACT ~24.3us (89%), DVE ~23.6us (86%) vs PE 27.3us
    (was ACT 97% / DVE 108% -> psum-recycle stalls)
  - warmup matmul count trimmed 14 -> 10
"""

import numpy as np
import ml_dtypes
from contextlib import ExitStack

import concourse.bass as bass
import concourse.bacc as bacc
import concourse.mybir as mybir
import concourse.tile as tile
from concourse.bass_utils import run_bass_kernel_spmd

B, S, H, D = 1, 4096, 16, 128
N_CORES = 8
HPC = H // N_CORES
SB = 1024
NSB = S // SB
NKT = S // 128
SCALE = float(1.0 / np.sqrt(D))
BF16 = mybir.dt.bfloat16
FP32 = mybir.dt.float32
I16 = mybir.dt.int16

SCH_SIGMA = 0.05754
SCH_A = float(SCALE * 128.0 / np.log(2.0))
SCH_B = float(128.0 * (127.0 - SCH_SIGMA))
DVE_SET = frozenset((2, 5, 8, 11, 14, 17, 20, 23, 26, 29))

_CACHE = {}


def _build():
    nc = bacc.Bacc("TRN2", target_bir_lowering=False, debug=False)
    qt_d = nc.dram_tensor("qt", [HPC, 128, S], BF16, kind="ExternalInput")
    kt_d = nc.dram_tensor("kt", [HPC, 128, S], BF16, kind="ExternalInput")
    vp_d = nc.dram_tensor("vp", [HPC, 128, S], BF16, kind="ExternalInput")
    o_d = nc.dram_tensor("o", [HPC, NSB, 128, SB], FP32, kind="ExternalOutput")
    lr_d = nc.dram_tensor("lr", [HPC, NSB, 128, 16 * SB], BF16, kind="ExternalOutput")

    with ExitStack() as ctx:
        tc = ctx.enter_context(tile.TileContext(nc))
        qkv = ctx.enter_context(tc.tile_pool(name="qkv", bufs=2))
        ptp = ctx.enter_context(tc.tile_pool(name="ptp", bufs=1))
        trp = ctx.enter_context(tc.tile_pool(name="trp", bufs=1))
        drp = ctx.enter_context(tc.tile_pool(name="drp", bufs=2))

        scp = ctx.enter_context(tc.tile_pool(name="scp", bufs=3, space="PSUM"))
        otp = ctx.enter_context(tc.tile_pool(name="otp", bufs=1, space="PSUM"))

        wsrc = qkv.tile([128, 512], BF16, name="wsrc", tag="wsrc")
        nc.vector.memset(wsrc, 1.0)
        wsc = scp.tile([128, SB], FP32, name="wsc", tag="sc")
        for wi in range(10):
            nc.tensor.matmul(wsc[:, (wi % 2) * 512:(wi % 2) * 512 + 512],
                             wsrc[:, :128], wsrc, start=True, stop=True)

        deferred = []
        pvq = []
        for h in range(HPC):
            qt_s = qkv.tile([128, S], BF16, name=f"qt{h}", tag="qt")
            kt_s = qkv.tile([128, S], BF16, name=f"kt{h}", tag="kt")
            v_s = qkv.tile([128, S], BF16, name=f"v{h}", tag="v")
            if h == 0:
                # need-ordered startup: kt k-tile0 + first q half, then rest
                nc.sync.dma_start(kt_s[:, 0:128], kt_d[h][:, 0:128])
                nc.sync.dma_start(qt_s[:, 0:512], qt_d[h][:, 0:512])
                nc.sync.dma_start(kt_s[:, 128:512], kt_d[h][:, 128:512])
                nc.sync.dma_start(qt_s[:, 512:1024], qt_d[h][:, 512:1024])
                nc.sync.dma_start(v_s[:, 0:512], vp_d[h][:, 0:512])
                bounds = [512, 1024, 2048, 3072, 4096]
                for a, b in zip(bounds[:-1], bounds[1:]):
                    nc.sync.dma_start(kt_s[:, a:b], kt_d[h][:, a:b])
                    if a >= 1024:
                        nc.sync.dma_start(qt_s[:, a:b], qt_d[h][:, a:b])
                    nc.sync.dma_start(v_s[:, a:b], vp_d[h][:, a:b])
            else:
                for a, b in [(0, 1024), (1024, 2048), (2048, 3072), (3072, 4096)]:
                    nc.sync.dma_start(kt_s[:, a:b], kt_d[h][:, a:b])
                    nc.sync.dma_start(qt_s[:, a:b], qt_d[h][:, a:b])
                    nc.sync.dma_start(v_s[:, a:b], vp_d[h][:, a:b])

            for sb in range(NSB):
                q0 = sb * SB
                last = (h == HPC - 1) and (sb == NSB - 1)
                ot = otp.tile([128, SB], FP32, name=f"ot_{h}_{sb}", tag="ot")
                pt = ptp.tile([128, NKT * SB], BF16, name=f"pt_{h}_{sb}", tag="pt")
                pt_i16 = pt.bitcast(I16)
                t1 = trp.tile([128, 16 * SB], BF16, name=f"t1_{h}_{sb}", tag="t1")

                def pv(j, ot=ot, pt=pt, v_s=v_s):
                    vj = v_s[:, j * 128:(j + 1) * 128]
                    pj = pt[:, j * SB:(j + 1) * SB]
                    nc.tensor.matmul(ot[:, :512], vj, pj[:, :512],
                                     start=(j == 0), stop=(j == NKT - 1))
                    nc.tensor.matmul(ot[:, 512:], vj, pj[:, 512:],
                                     start=(j == 0), stop=(j == NKT - 1))

                def l0half(c, hh2, pt=pt, t1=t1):
                    # one L0 add: 4 pt tiles -> 2 t1 tiles for chunk c half hh2
                    o2 = (2 * c + hh2) * 4 * SB
                    src = pt[:, o2:o2 + 4 * SB].rearrange(
                        "p (t two q) -> p t two q", two=2, q=SB)
                    dst = t1[:, o2 // 2:o2 // 2 + 2 * SB].rearrange(
                        "p (t q) -> p t q", q=SB)
                    nc.vector.tensor_add(dst, src[:, :, 0, :], src[:, :, 1, :])

                def lrq(c, h=h, sb=sb, t1=t1, last=last):
                    # DMA out lr chunk c ( = t1 tiles 4c..4c+3 )
                    if last:
                        for qq in range(4):
                            c2 = slice((4 * c + qq) * SB, (4 * c + qq + 1) * SB)
                            nc.sync.dma_start(lr_d[h, sb][:, c2], t1[:, c2])
                    else:
                        cs = slice(c * 4 * SB, (c + 1) * 4 * SB)
                        nc.sync.dma_start(lr_d[h, sb][:, cs], t1[:, cs])

                for j in range(NKT):
                    sc = scp.tile([128, SB], FP32, name=f"sc_{h}_{sb}_{j}", tag="sc")
                    kj = kt_s[:, j * 128:(j + 1) * 128]
                    nc.tensor.matmul(sc[:, :512], kj, qt_s[:, q0:q0 + 512],
                                     start=True, stop=True)
                    nc.tensor.matmul(sc[:, 512:], kj, qt_s[:, q0 + 512:q0 + SB],
                                     start=True, stop=True)
                    if j in DVE_SET:
                        nc.vector.tensor_scalar(
                            pt_i16[:, j * SB:(j + 1) * SB], sc, SCH_A, SCH_B,
                            mybir.AluOpType.mult, mybir.AluOpType.add)
                    else:
                        nc.scalar.activation(
                            pt[:, j * SB:(j + 1) * SB], sc,
                            mybir.ActivationFunctionType.Exp, scale=SCALE)

                    # L0 tree beats on DVE; lr chunk DMA as soon as ready
                    if j == 9:
                        l0half(0, 0)
                    elif j == 10:
                        l0half(0, 1)
                    elif j == 12:
                        lrq(0)
                    elif j == 17:
                        l0half(1, 0)
                    elif j == 18:
                        l0half(1, 1)
                    elif j == 20:
                        lrq(1)
                    elif j == 25:
                        l0half(2, 0)
                    elif j == 27:
                        l0half(2, 1)
                    elif j == 29:
                        lrq(2)
                    if deferred and j in (1, 3, 5):
                        deferred.pop(0)()

                    def pvstep(j=j, pv=pv, h=h, sb=sb, ot=ot, last=last):
                        pv(j)
                        if j == NKT - 1:
                            # sb epilogue rides with the last PV: drain ot
                            osb = drp.tile([128, SB], FP32,
                                           name=f"osb_{h}_{sb}", tag="osb")
                            for qq in range(4):
                                cs = slice(qq * SB // 4, (qq + 1) * SB // 4)
                                if last:
                                    nc.scalar.copy(osb[:, cs], ot[:, cs])
                                else:
                                    nc.vector.tensor_copy(osb[:, cs], ot[:, cs])
                                nc.sync.dma_start(o_d[h, sb][:, cs], osb[:, cs])
                    pvq.append(pvstep)
                    if len(pvq) > 3:
                        pvq.pop(0)()

                def tailc3a(l0half=l0half):
                    l0half(3, 0)
                def tailc3b(l0half=l0half):
                    l0half(3, 1)
                def taillr(lrq=lrq):
                    lrq(3)
                if last:
                    while pvq:
                        pvq.pop(0)()
                    tailc3a(); tailc3b(); taillr()
                else:
                    deferred.extend([tailc3a, tailc3b, taillr])
        while pvq:
            pvq.pop(0)()
        while deferred:
            deferred.pop(0)()
    nc.compile()
    return nc


def _prep_inputs(q, k, v):
    bf = ml_dtypes.bfloat16
    in_maps = []
    for c in range(N_CORES):
        hs = slice(c * HPC, (c + 1) * HPC)
        qt = np.transpose(q[:, hs, :], (1, 2, 0)).astype(bf)
        kt = np.transpose(k[:, hs, :], (1, 2, 0)).astype(bf)
        vh = np.transpose(v[:, hs, :], (1, 0, 2))
        vp = np.ascontiguousarray(
            vh.reshape(HPC, S // 128, 128, D).transpose(0, 2, 1, 3)
        ).reshape(HPC, 128, S).astype(bf)
        in_maps.append({"qt": qt, "kt": kt, "vp": vp})
    return in_maps


def kernel(q, k, v, ring_size=None, **_unused):
    q = np.asarray(q, dtype=np.float32).reshape(S, H, D)
    k = np.asarray(k, dtype=np.float32).reshape(S, H, D)
    v = np.asarray(v, dtype=np.float32).reshape(S, H, D)

    in_maps = _prep_inputs(q, k, v)
    if "nc" not in _CACHE:
        _CACHE["nc"] = _build()
    res = run_bass_kernel_spmd(_CACHE["nc"], in_maps, list(range(N_CORES))).results

    out = np.empty((B, S, H, D), np.float32)
    for c in range(N_CORES):
        o = np.asarray(res[c]["o"])
        lr = np.asarray(res[c]["lr"]).astype(np.float32)
        for hh in range(HPC):
            l = lr[hh].reshape(NSB, 128, 16, SB).sum(axis=(1, 2))
            on = o[hh] / l[:, None, :]
            out[0, :, c * HPC + hh, :] = on.transpose(0, 2, 1).reshape(S, D)
    return out
